# revision 24
# baseline (speedup 1.0000x reference)
"""BERT self-attention (B=16, T=512, C=768, H=12, D=64) on 8 trn2 NeuronCores.

Data-parallel over batch: each core gets 2 batches. Matmul operands fp16,
fp32 PSUM accumulation. Per core:
  inputs shipped in per-partition-contiguous [128, X] layouts (host
         pre-transposes x and pre-shuffles weights) so every input DMA is
         a full-burst contiguous transfer; W_qk lands n-major so the QK
         chain starts as soon as one 192KB slice + xT-b0 arrive.
  warmup zero-matmuls run during the preamble/DMA window so HAM
         un-throttles the PE clock before real matmuls start.
  S^T    = K^T-as-lhsT matmul -> [key, query]; two heads of a pair go to
         separate PSUM banks at row positions 0/64 (row-split packing).
         S for head-pair hp+1 is emitted BEFORE PV of pair hp so the PE's
         strict-FIFO queue never head-of-line blocks on exp.
  P      = exp(S/8 + mask) on ScalarE, fp16. ScalarE runs only Exp and
         table-free Identity/Copy -> no ACT_TABLE_LOAD thrash.
  y^T    = lhsT=[V_h | ones](padded to 128 cols for FWL) matmul ->
         unnormalized y^T + row sums in PSUM row 64; sums staged fp16
         (same-base ScalarE copy), DMA-relocated to partitions 0/1, one
         K=2 matmul vs a 0/1 pattern replicates them across partitions,
         reciprocal_approx_fast on VectorE, then normalize straight out
         of PSUM (VectorE rows 0-63, GpSimd rows 64-127).
  out    = y^T-as-lhsT matmul + bias add -> fp16 staged tile, DMA out.
"""

import sys

sys.path.insert(0, "/opt/trn_rl_repo")

from contextlib import ExitStack

import numpy as np

B, T, C = 16, 512, 768
H, D = 12, 64
N_CORES = 8
BC = B // N_CORES           # batches per core
M = BC * T                  # tokens per core
KT = C // 128               # feature k-tiles (6)
TT = M // 128               # token tiles per core (8)
NQK = 2 * C // 128          # q+k feature n-tiles (12)
VW = H * 65 + 63            # v tile: per-head [V_h | ones] + FWL pad
SCALE = 1.0 / np.sqrt(D)

_cache = {}


def _build():
    import concourse.bass as bass
    import concourse.tile as tile
    from concourse import bacc, mybir
    f32 = mybir.dt.float32
    f16 = mybir.dt.float16
    Exp = mybir.ActivationFunctionType.Exp
    Ident = mybir.ActivationFunctionType.Identity
    Add = mybir.AluOpType.add
    Mult = mybir.AluOpType.mult

    J_OF = {n: j for j, n in enumerate(
        (0, 6, 1, 7, 2, 8, 3, 9, 4, 10, 5, 11))}
    nc = bacc.Bacc("TRN2", target_bir_lowering=False, debug=False,
                   num_devices=N_CORES)
    # all inputs per-partition-contiguous [128, X]
    xt_d = nc.dram_tensor("xt", [128, BC * KT * T], f16,
                          kind="ExternalInput").ap()
    wqk_d = nc.dram_tensor("wqk", [128, NQK * KT * 128], f16,
                           kind="ExternalInput").ap()
    wv_d = nc.dram_tensor("wv", [128, KT * C], f16,
                          kind="ExternalInput").ap()
    wp_d = nc.dram_tensor("wp", [128, KT * C], f16,
                          kind="ExternalInput").ap()
    qm_d = nc.dram_tensor("qm32", [128, 20], f32, kind="ExternalInput").ap()
    bb_d = nc.dram_tensor("bb32", [1, 2 * C], f32, kind="ExternalInput").ap()
    e2_d = nc.dram_tensor("e2pat", [2, 128], f16, kind="ExternalInput").ap()
    out_d = nc.dram_tensor("out", [M, C], f16, kind="ExternalOutput").ap()

    with tile.TileContext(nc) as tc, ExitStack() as ctx:
        pp = ctx.enter_context(tc.tile_pool(name="pp", bufs=1))
        np_ = ctx.enter_context(tc.tile_pool(name="norm", bufs=4))
        ap_ = ctx.enter_context(tc.tile_pool(name="att", bufs=13))
        ps_mm = ctx.enter_context(tc.tile_pool(name="ps_mm", bufs=2, space="PSUM"))
        ps_s = ctx.enter_context(tc.tile_pool(name="ps_s", bufs=2, space="PSUM"))
        ps_y = ctx.enter_context(tc.tile_pool(name="ps_y", bufs=2, space="PSUM"))

        # ---- static tiles ----
        qm = pp.tile([128, 20], f32, tag="qm")
        ba_qk = qm[:, 0:NQK]
        mask_sb = qm[:, NQK:NQK + BC * 4]
        bb = pp.tile([128, 2 * C], f32, tag="bb")
        ba_v_rep = bb[:, 0:C]
        bp_rep = bb[:, C:2 * C]
        e2t = pp.tile([2, 128], f16, tag="e2")
        wqk_t = pp.tile([128, NQK, KT, 128], f16, tag="wqk")
        wv_a = pp.tile([128, KT, 512], f16, tag="wv_a")
        wv_b = pp.tile([128, KT, 256], f16, tag="wv_b")
        wp_all = pp.tile([128, KT, C], f16, tag="wp")
        wp_t = [wp_all[:, k, :] for k in range(KT)]
        xt_t = pp.tile([128, BC, KT, T], f16, tag="xT")
        v_t = [pp.tile([128, VW], f16, tag=f"v{t}", name=f"v{t}")
               for t in range(TT)]
        warm = pp.tile([128, 512], f16, tag="warm")

        # ---- b0-critical input DMAs on the SWDGE path (3rd queue) ----
        nc.gpsimd.dma_start(xt_t[:, 0, :, :], xt_d[:, 0:KT * T])
        nc.gpsimd.dma_start(wv_a[:], wv_d[:, 0:KT * 512])
        nc.gpsimd.dma_start(wv_b[:], wv_d[:, KT * 512:KT * C])

        # ---- PE warmup during the preamble/DMA window ----
        nc.vector.memset(warm[:], 0.0)
        for i in range(14):
            pw = ps_mm.tile([128, 512], f32, tag="mm", name=f"warm{i}")
            nc.tensor.matmul(pw[:], warm[:, 0:128], warm[:],
                             start=True, stop=True)
        for t in range(TT):
            nc.gpsimd.memset(
                v_t[t][:, 0:H * 65].rearrange(
                    "p (h c) -> p h c", c=65)[:, :, 64:65],
                1.0)
            nc.gpsimd.memset(v_t[t][:, H * 65:VW], 0.0)

        # ---- input DMAs (all contiguous [128, cols] slices), in the order
        # the interleaved chain+attention consumes them ----
        nc.scalar.dma_start(qm[:], qm_d[:])
        # wqk shipped in consumption order [0,6 | 1,7,2,8 | 3,9,4,10,5,11]
        nc.scalar.dma_start(wqk_t[:, 0:2, :, :],
                            wqk_d[:, 0:2 * KT * 128])
        nc.scalar.dma_start(wqk_t[:, 2:6, :, :],
                            wqk_d[:, 2 * KT * 128:6 * KT * 128])
        nc.scalar.dma_start(wqk_t[:, 6:NQK, :, :],
                            wqk_d[:, 6 * KT * 128:NQK * KT * 128])
        nc.scalar.dma_start(bb[:], bb_d[0:1, :].partition_broadcast(128))
        nc.scalar.dma_start(e2t[:], e2_d[:])

        ones_r = pp.tile([1, 128], f16, tag="ones_r")
        nc.vector.memset(ones_r[:], 1.0)
        bp16 = pp.tile([1, C], f16, tag="bp16")
        nc.vector.tensor_copy(bp16[:], bb[0:1, C:2 * C])

        qkT = [pp.tile([128, M], f16, tag=f"qk{n}", name=f"qk{n}")
               for n in range(NQK)]
        yT_t = [pp.tile([128, M], f16, tag=f"yT{c}", name=f"yT{c}")
                for c in range(KT)]

        def qkv_chain(b, i):
            """i in [0, 20): 12 QK n-tiles then 8 V half-tiles."""
            bcol = b * T
            if i < NQK:
                n = i
                p = ps_mm.tile([128, 512], f32, tag="mm", name=f"mm{b}_{i}")
                for k in range(KT):
                    nc.tensor.matmul(
                        p[:],
                        wqk_t[:, J_OF[n], k, :],
                        xt_t[:, b, k, :],
                        start=(k == 0), stop=(k == KT - 1))
                if b == 0:
                    nc.scalar.activation(
                        qkT[n][:, bcol:bcol + T], p[:], Ident,
                        bias=ba_qk[:, n:n + 1])
                else:
                    nc.vector.tensor_scalar_add(
                        qkT[n][:, bcol:bcol + T], p[:], ba_qk[:, n:n + 1])
            else:
                j = i - NQK
                t4 = j // 2
                t = b * 4 + t4
                lo, w = ((0, 512), (512, 256))[j % 2]
                wv_h = wv_a if j % 2 == 0 else wv_b
                p = ps_mm.tile([128, 512], f32, tag="mm", name=f"mm{b}_{i}")
                for k in range(KT):
                    nc.tensor.matmul(
                        p[:, :w],
                        xt_t[:, b, k, t4 * 128:(t4 + 1) * 128],
                        wv_h[:, k, 0:w],
                        start=(k == 0), stop=(k == KT - 1))
                h0 = lo // D
                nc.vector.tensor_tensor(
                    out=v_t[t][:, 0:H * 65].rearrange(
                        "p (h c) -> p h c", c=65)[:, h0:h0 + w // D, 0:64],
                    in0=p[:, :w].rearrange("p (h c) -> p h c", c=D),
                    in1=ba_v_rep[:, lo:lo + w].rearrange(
                        "p (h c) -> p h c", c=D),
                    op=Add)

        e_hp = {}
        py_tiles = {}
        pair_tiles = {}
        srow_tiles = {}

        def attention_S(b, hp):
            """S^T matmuls + exp for head-pair hp: fills e_hp[(b, hp)]."""
            bcol = b * T
            e_tiles = []
            for kt in range(4):
                ps = ps_s.tile([128, 1024], f32)
                for sub in range(2):
                    r0 = 64 * sub
                    nc.tensor.matmul(
                        ps[:, sub * 512:sub * 512 + 512],
                        qkT[6 + hp][r0:r0 + D,
                                    bcol + kt * 128:bcol + (kt + 1) * 128],
                        qkT[hp][r0:r0 + D, bcol:bcol + T],
                        start=True, stop=True)
                e = ap_.tile([128, 1024], f16, tag="e")
                nc.scalar.activation(
                    e[:], ps[:], Exp,
                    bias=mask_sb[:, b * 4 + kt:b * 4 + kt + 1],
                    scale=float(SCALE))
                e_tiles.append(e)
            e_hp[(b, hp)] = e_tiles

        def attention_PV(b, hp):
            """PV matmuls + denominator staging for head-pair hp."""
            e_tiles = e_hp.pop((b, hp))
            rs = np_.tile([65, 1024], f16, tag="rstage", bufs=4,
                          name=f"rs{b}_{hp}")
            srow = np_.tile([2, 512], f16, tag="srow", bufs=4,
                            name=f"srow{b}_{hp}")
            srow_tiles[(b, hp)] = srow
            pair = np_.tile([128, 512], f16, tag="pair", bufs=4,
                            name=f"pair{b}_{hp}")
            pair_tiles[(b, hp)] = pair
            for sub in range(2):
                h = 2 * hp + sub
                py = ps_y.tile([128, 512], f32)
                py_tiles[(b, hp, sub)] = py
                for kt in range(4):
                    nc.tensor.matmul(
                        py[:, :],
                        v_t[b * 4 + kt][:, 65 * h:65 * h + 128],
                        e_tiles[kt][:, sub * 512:sub * 512 + 512],
                        start=(kt == 0), stop=(kt == 3))
                # stage softmax denominators (same-base copy, fp16 cast)
                nc.vector.tensor_copy(
                    rs[64:65, sub * 512:sub * 512 + 512], py[64:65, :])
                # relocate to partition `sub` for the K=2 replicate matmul
                nc.scalar.dma_start(
                    srow[sub:sub + 1, :],
                    rs[64:65, sub * 512:sub * 512 + 512])
                if sub == 1:
                    st = np_.tile([64, 512], f16, tag="stage")
                    nc.vector.tensor_copy(st[:], py[0:64, :])
                    nc.scalar.dma_start(pair[64:128, :], st[:])

        def norm_apply(b, hp, tc_lo=0, tc_hi=4, split=False):
            """Normalize head-pair hp (k-tile hp) of batch b."""
            bcol = b * T
            srow = srow_tiles[(b, hp)]
            if tc_lo == 0:
                rep = ps_mm.tile([128, 512], f32, tag="mm",
                                 name=f"rep{b}_{hp}")
                nc.tensor.matmul(rep[:], e2t[:], srow[:],
                                 start=True, stop=True)
                rcp = np_.tile([128, 512], f32, tag="rcp", bufs=3,
                               name=f"rcp{b}_{hp}")
                srow_tiles[(b, hp, "rcp")] = rcp
                nc.vector.reciprocal_approx_fast(rcp[:], rep[:])
            else:
                rcp = srow_tiles[(b, hp, "rcp")]
            py0 = py_tiles[(b, hp, 0)]
            pair = pair_tiles[(b, hp)]
            lo = tc_lo * 128
            w = (tc_hi - tc_lo) * 128
            nc.vector.tensor_tensor(
                out=yT_t[hp][0:64, bcol + lo:bcol + lo + w],
                in0=py0[0:64, lo:lo + w],
                in1=rcp[0:64, lo:lo + w],
                op=Mult)
            nc.gpsimd.tensor_tensor(
                out=yT_t[hp][64:128, bcol + lo:bcol + lo + w],
                in0=pair[64:128, lo:lo + w],
                in1=rcp[64:128, lo:lo + w],
                op=Mult)

        pj_part = {}
        ot_tiles = {}

        def proj_chunk(b, i, ks=0, ke=KT, partial=False, tail=False):
            t = b * 4 + i // 2
            lo, w = ((0, 512), (512, 256))[i % 2]
            p = ps_mm.tile([128, 512], f32, tag="mm", name=f"pj{b}_{i}_{ks}")
            for k in range(ks, ke):
                nc.tensor.matmul(
                    p[:, :w],
                    yT_t[k][:, t * 128:(t + 1) * 128],
                    wp_t[k][:, lo:lo + w],
                    start=(k == ks), stop=(k == ke - 1 and not partial))
            if partial:
                nc.tensor.matmul(
                    p[:, :w], ones_r[0:1, :], bp16[0:1, lo:lo + w],
                    start=False, stop=True)
                pt = np_.tile([128, 512], f32, tag="pjpart", bufs=8,
                              name=f"pjpart{i}")
                nc.vector.tensor_copy(pt[:, :w], p[:, :w])
                pj_part[(b, i)] = pt
                return
            if tail:
                if i % 2 == 0:
                    ot = np_.tile([128, C], f16, tag="otail", bufs=4,
                                  name=f"ott{i}")
                    ot_tiles[(b, t)] = ot
                else:
                    ot = ot_tiles[(b, t)]
                off = lo
            else:
                if i % 4 == 0:
                    ot = np_.tile([128, 2 * C], f16, tag="ostage", bufs=3,
                                  name=f"ot{b}_{i}")
                    ot_tiles[(b, t // 2)] = ot
                else:
                    ot = ot_tiles[(b, t // 2)]
                off = (t % 2) * C + lo
            if (b, i) in pj_part:
                nc.vector.tensor_tensor(
                    out=ot[:, off:off + w], in0=p[:, :w],
                    in1=pj_part[(b, i)][:, :w], op=Add)
            else:
                nc.vector.tensor_tensor(
                    out=ot[:, off:off + w], in0=p[:, :w],
                    in1=bp_rep[:, lo:lo + w], op=Add)
            if tail:
                nc.sync.dma_start(out_d[t * 128:(t + 1) * 128, lo:lo + w],
                                  ot[:, lo:lo + w])
            elif not tail and i % 4 == 3:
                t0 = t - 1
                nc.sync.dma_start(
                    out_d[t0 * 128:(t0 + 2) * 128, :].rearrange(
                        "(t p) c -> p t c", p=128),
                    ot[:].rearrange("p (t c) -> p t c", c=C))

        # ---- software-pipelined emission ----
        # b0: attention is interleaved INTO the qkv chain so exps start
        # during the (HBM-bound) input window. Chain pair (n_hp, n_6+hp)
        # is emitted just before S(b, hp); V-512 chunks early (heads 0-7),
        # V-256 (heads 8-11) later. S leads PV by 2 pairs; norm lags PV
        # by 1 pair.
        B_ITEMS = ([0, 6, 12, 14], [1, 7, 16, 18], [2, 8, 13, 15],
                   [3, 9, 17, 19], [4, 10], [5, 11])
        for i in B_ITEMS[0]:
            qkv_chain(0, i)
        for i in B_ITEMS[1]:
            qkv_chain(0, i)
        attention_S(0, 0)
        attention_S(0, 1)
        nc.sync.dma_start(xt_t[:, 1, :, :], xt_d[:, KT * T:2 * KT * T])
        for hp in range(2, 6):
            if hp == 3:
                nc.sync.dma_start(wp_all[:], wp_d[:, 0:KT * C])
            for i in B_ITEMS[hp]:
                qkv_chain(0, i)
            attention_S(0, hp)
            attention_PV(0, hp - 2)
            if hp >= 3:
                norm_apply(0, hp - 3)
        # drain b0 while feeding the b1 chain
        for g in (0, 1):
            for i in B_ITEMS[g]:
                qkv_chain(1, i)
            attention_PV(0, 4 + g)
            norm_apply(0, 3 + g)
        attention_S(1, 0)
        attention_S(1, 1)
        pj0 = iter(range(8))
        for hp in range(6):
            if hp + 2 <= 5:
                for i in B_ITEMS[hp + 2]:
                    qkv_chain(1, i)
                attention_S(1, hp + 2)
            if hp == 0:
                norm_apply(0, 5)
            if hp == 5:
                # partial-proj first half runs on PE during exp(1,5)
                norm_apply(1, 4)
                for i in range(4):
                    proj_chunk(1, i, 0, 5, partial=True)
            attention_PV(1, hp)
            if 1 <= hp <= 4:
                norm_apply(1, hp - 1)
            for _ in range((1, 1, 1, 1, 2, 2)[hp]):
                i = next(pj0, None)
                if i is not None:
                    proj_chunk(0, i)
        for i in range(4, 8):
            proj_chunk(1, i, 0, 5, partial=True)
        for i in pj0:
            proj_chunk(0, i)
        # pipelined tail: per 128-token chunk, normalize then project k=5
        for tc in range(4):
            norm_apply(1, 5, tc, tc + 1)
            proj_chunk(1, 2 * tc, 5, KT, tail=True)
            proj_chunk(1, 2 * tc + 1, 5, KT, tail=True)

    nc.compile()
    return nc


def get_compiled():
    if "nc" not in _cache:
        _cache["nc"] = _build()
    return _cache["nc"]


def make_in_maps(x, attention_mask, W_attn, b_attn, W_proj, b_proj):
    x = np.asarray(x, dtype=np.float32).astype(np.float16)
    mask = np.asarray(attention_mask, dtype=np.float32)[:, 0, 0, :]
    wa = np.asarray(W_attn, dtype=np.float32).astype(np.float16)
    ba = np.asarray(b_attn, dtype=np.float32)
    wp = np.asarray(W_proj, dtype=np.float32).astype(np.float16)
    bp = np.asarray(b_proj, dtype=np.float32)
    bb = np.ascontiguousarray(
        np.concatenate([ba[2 * C:], bp]).reshape(1, 2 * C))
    # per-partition-contiguous layouts: row p holds that partition's data
    # wqk[p, j, k, c] = W_attn[k*128+p, n*128+c], j = consumption order
    NORD = (0, 6, 1, 7, 2, 8, 3, 9, 4, 10, 5, 11)
    wqk = np.ascontiguousarray(
        wa[:, :2 * C].reshape(KT, 128, NQK, 128).transpose(
            1, 2, 0, 3)[:, NORD].reshape(128, NQK * KT * 128))
    # wv: [p, half0: k x 512 | half1: k x 256]
    wv3 = wa[:, 2 * C:].reshape(KT, 128, C).transpose(1, 0, 2)
    wv = np.ascontiguousarray(np.concatenate(
        [wv3[:, :, 0:512].reshape(128, KT * 512),
         wv3[:, :, 512:C].reshape(128, KT * 256)], axis=1))
    wpp = np.ascontiguousarray(
        wp.reshape(KT, 128, C).transpose(1, 0, 2).reshape(128, KT * C))
    e2 = np.zeros((2, 128), dtype=np.float16)
    e2[0, 0:64] = 1.0
    e2[1, 64:128] = 1.0
    maps = []
    for i in range(N_CORES):
        qm = np.zeros((128, 20), dtype=np.float32)
        qm[:, :NQK] = ba[0:2 * C].reshape(NQK, 128).T
        qm[:, NQK:] = mask[BC * i:BC * (i + 1)].reshape(-1).reshape(
            BC * 4, 128).T
        # xt[p, b, k, m] = x[core, b, m, k*128+p]
        xc = x[BC * i:BC * (i + 1)]                      # [BC, T, C]
        xt = np.ascontiguousarray(
            xc.reshape(BC, T, KT, 128).transpose(3, 0, 2, 1).reshape(
                128, BC * KT * T))
        maps.append({
            "xt": xt, "qm32": qm, "bb32": bb,
            "wqk": wqk, "wv": wv, "wp": wpp, "e2pat": e2,
        })
    return maps


def kernel(x, attention_mask, W_attn, b_attn, W_proj, b_proj):
    from concourse.bass_utils import run_bass_kernel_spmd

    nc = get_compiled()
    in_maps = make_in_maps(x, attention_mask, W_attn, b_attn, W_proj, b_proj)
    last_err = None
    for _ in range(3):
        try:
            res = run_bass_kernel_spmd(nc, in_maps, list(range(N_CORES)))
            break
        except Exception as e:  # transient NRT device errors: retry
            last_err = e
    else:
        raise last_err
    out = np.concatenate(
        [res.results[i]["out"].reshape(BC, T, C) for i in range(N_CORES)],
        axis=0)
    return out.astype(np.float32)


# revision 25
# speedup vs baseline: 1.0107x; 1.0107x over previous
"""BERT self-attention (B=16, T=512, C=768, H=12, D=64) on 8 trn2 NeuronCores.

Data-parallel over batch: each core gets 2 batches. Matmul operands fp16,
fp32 PSUM accumulation. Per core:
  inputs shipped in per-partition-contiguous [128, X] layouts (host
         pre-transposes x and pre-shuffles weights) so every input DMA is
         a full-burst contiguous transfer; W_qk lands n-major so the QK
         chain starts as soon as one 192KB slice + xT-b0 arrive.
  warmup zero-matmuls run during the preamble/DMA window so HAM
         un-throttles the PE clock before real matmuls start.
  S^T    = K^T-as-lhsT matmul -> [key, query]; two heads of a pair go to
         separate PSUM banks at row positions 0/64 (row-split packing).
         S for head-pair hp+1 is emitted BEFORE PV of pair hp so the PE's
         strict-FIFO queue never head-of-line blocks on exp.
  P      = exp(S/8 + mask) on ScalarE, fp16. ScalarE runs only Exp and
         table-free Identity/Copy -> no ACT_TABLE_LOAD thrash.
  y^T    = lhsT=[V_h | ones](padded to 128 cols for FWL) matmul ->
         unnormalized y^T + row sums in PSUM row 64; sums staged fp16
         (same-base ScalarE copy), DMA-relocated to partitions 0/1, one
         K=2 matmul vs a 0/1 pattern replicates them across partitions,
         reciprocal_approx_fast on VectorE, then normalize straight out
         of PSUM (VectorE rows 0-63, GpSimd rows 64-127).
  out    = y^T-as-lhsT matmul + bias add -> fp16 staged tile, DMA out.
"""

import sys

sys.path.insert(0, "/opt/trn_rl_repo")

from contextlib import ExitStack

import numpy as np

B, T, C = 16, 512, 768
H, D = 12, 64
N_CORES = 8
BC = B // N_CORES           # batches per core
M = BC * T                  # tokens per core
KT = C // 128               # feature k-tiles (6)
TT = M // 128               # token tiles per core (8)
NQK = 2 * C // 128          # q+k feature n-tiles (12)
VW = H * 65 + 63            # v tile: per-head [V_h | ones] + FWL pad
SCALE = 1.0 / np.sqrt(D)

_cache = {}


def _build():
    import concourse.bass as bass
    import concourse.tile as tile
    from concourse import bacc, mybir
    f32 = mybir.dt.float32
    f16 = mybir.dt.float16
    Exp = mybir.ActivationFunctionType.Exp
    Ident = mybir.ActivationFunctionType.Identity
    Add = mybir.AluOpType.add
    Mult = mybir.AluOpType.mult

    J_OF = {n: j for j, n in enumerate(
        (0, 6, 1, 7, 2, 8, 3, 9, 4, 10, 5, 11))}
    nc = bacc.Bacc("TRN2", target_bir_lowering=False, debug=False,
                   num_devices=N_CORES)
    # all inputs per-partition-contiguous [128, X]
    xt_d = nc.dram_tensor("xt", [128, BC * KT * T], f16,
                          kind="ExternalInput").ap()
    wqk_d = nc.dram_tensor("wqk", [128, NQK * KT * 128], f16,
                           kind="ExternalInput").ap()
    wv_d = nc.dram_tensor("wv", [128, KT * C], f16,
                          kind="ExternalInput").ap()
    wp_d = nc.dram_tensor("wp", [128, KT * C], f16,
                          kind="ExternalInput").ap()
    qm_d = nc.dram_tensor("qm32", [128, 20], f32, kind="ExternalInput").ap()
    bb_d = nc.dram_tensor("bb32", [1, 2 * C], f32, kind="ExternalInput").ap()
    e2_d = nc.dram_tensor("e2pat", [2, 128], f16, kind="ExternalInput").ap()
    out_d = nc.dram_tensor("out", [M, C], f16, kind="ExternalOutput").ap()

    with tile.TileContext(nc) as tc, ExitStack() as ctx:
        pp = ctx.enter_context(tc.tile_pool(name="pp", bufs=1))
        np_ = ctx.enter_context(tc.tile_pool(name="norm", bufs=4))
        ap_ = ctx.enter_context(tc.tile_pool(name="att", bufs=13))
        ps_mm = ctx.enter_context(tc.tile_pool(name="ps_mm", bufs=2, space="PSUM"))
        ps_s = ctx.enter_context(tc.tile_pool(name="ps_s", bufs=2, space="PSUM"))
        ps_y = ctx.enter_context(tc.tile_pool(name="ps_y", bufs=2, space="PSUM"))

        # ---- static tiles ----
        qm = pp.tile([128, 20], f32, tag="qm")
        ba_qk = qm[:, 0:NQK]
        mask_sb = qm[:, NQK:NQK + BC * 4]
        bb = pp.tile([128, 2 * C], f32, tag="bb")
        ba_v_rep = bb[:, 0:C]
        bp_rep = bb[:, C:2 * C]
        e2t = pp.tile([2, 128], f16, tag="e2")
        wqk_t = pp.tile([128, NQK, KT, 128], f16, tag="wqk")
        wv_a = pp.tile([128, KT, 512], f16, tag="wv_a")
        wv_b = pp.tile([128, KT, 256], f16, tag="wv_b")
        wp_all = pp.tile([128, KT, C], f16, tag="wp")
        wp_t = [wp_all[:, k, :] for k in range(KT)]
        xt_t = pp.tile([128, BC, KT, T], f16, tag="xT")
        v_t = [pp.tile([128, VW], f16, tag=f"v{t}", name=f"v{t}")
               for t in range(TT)]
        warm = pp.tile([128, 512], f16, tag="warm")

        # ---- b0-critical input DMAs on the SWDGE path (3rd queue) ----
        nc.gpsimd.dma_start(xt_t[:, 0, :, :], xt_d[:, 0:KT * T])
        nc.gpsimd.dma_start(wv_a[:], wv_d[:, 0:KT * 512])
        nc.gpsimd.dma_start(wv_b[:], wv_d[:, KT * 512:KT * C])

        # ---- PE warmup during the preamble/DMA window ----
        nc.vector.memset(warm[:], 0.0)
        for i in range(14):
            pw = ps_mm.tile([128, 512], f32, tag="mm", name=f"warm{i}")
            nc.tensor.matmul(pw[:], warm[:, 0:128], warm[:],
                             start=True, stop=True)
        for t in range(TT):
            nc.gpsimd.memset(
                v_t[t][:, 0:H * 65].rearrange(
                    "p (h c) -> p h c", c=65)[:, :, 64:65],
                1.0)
            nc.gpsimd.memset(v_t[t][:, H * 65:VW], 0.0)

        # ---- input DMAs (all contiguous [128, cols] slices), in the order
        # the interleaved chain+attention consumes them ----
        nc.scalar.dma_start(qm[:], qm_d[:])
        # wqk shipped in consumption order [0,6 | 1,7,2,8 | 3,9,4,10,5,11]
        nc.scalar.dma_start(wqk_t[:, 0:2, :, :],
                            wqk_d[:, 0:2 * KT * 128])
        nc.scalar.dma_start(wqk_t[:, 2:6, :, :],
                            wqk_d[:, 2 * KT * 128:6 * KT * 128])
        nc.scalar.dma_start(wqk_t[:, 6:NQK, :, :],
                            wqk_d[:, 6 * KT * 128:NQK * KT * 128])
        nc.scalar.dma_start(bb[:], bb_d[0:1, :].partition_broadcast(128))
        nc.scalar.dma_start(e2t[:], e2_d[:])

        ones_r = pp.tile([1, 128], f16, tag="ones_r")
        nc.vector.memset(ones_r[:], 1.0)
        bp16 = pp.tile([1, C], f16, tag="bp16")
        nc.vector.tensor_copy(bp16[:], bb[0:1, C:2 * C])

        qkT = [pp.tile([128, M], f16, tag=f"qk{n}", name=f"qk{n}")
               for n in range(NQK)]
        yT_t = [pp.tile([128, M], f16, tag=f"yT{c}", name=f"yT{c}")
                for c in range(KT)]

        def qkv_chain(b, i):
            """i in [0, 20): 12 QK n-tiles then 8 V half-tiles."""
            bcol = b * T
            if i < NQK:
                n = i
                p = ps_mm.tile([128, 512], f32, tag="mm", name=f"mm{b}_{i}")
                for k in range(KT):
                    nc.tensor.matmul(
                        p[:],
                        wqk_t[:, J_OF[n], k, :],
                        xt_t[:, b, k, :],
                        start=(k == 0), stop=(k == KT - 1))
                if b == 0:
                    nc.scalar.activation(
                        qkT[n][:, bcol:bcol + T], p[:], Ident,
                        bias=ba_qk[:, n:n + 1])
                else:
                    nc.vector.tensor_scalar_add(
                        qkT[n][:, bcol:bcol + T], p[:], ba_qk[:, n:n + 1])
            else:
                j = i - NQK
                t4 = j // 2
                t = b * 4 + t4
                lo, w = ((0, 512), (512, 256))[j % 2]
                wv_h = wv_a if j % 2 == 0 else wv_b
                p = ps_mm.tile([128, 512], f32, tag="mm", name=f"mm{b}_{i}")
                for k in range(KT):
                    nc.tensor.matmul(
                        p[:, :w],
                        xt_t[:, b, k, t4 * 128:(t4 + 1) * 128],
                        wv_h[:, k, 0:w],
                        start=(k == 0), stop=(k == KT - 1))
                h0 = lo // D
                nc.vector.tensor_tensor(
                    out=v_t[t][:, 0:H * 65].rearrange(
                        "p (h c) -> p h c", c=65)[:, h0:h0 + w // D, 0:64],
                    in0=p[:, :w].rearrange("p (h c) -> p h c", c=D),
                    in1=ba_v_rep[:, lo:lo + w].rearrange(
                        "p (h c) -> p h c", c=D),
                    op=Add)

        e_hp = {}
        py_tiles = {}
        pair_tiles = {}
        srow_tiles = {}

        def attention_S(b, hp):
            """S^T matmuls + exp for head-pair hp: fills e_hp[(b, hp)]."""
            bcol = b * T
            e_tiles = []
            for kt in range(4):
                ps = ps_s.tile([128, 1024], f32)
                for sub in range(2):
                    r0 = 64 * sub
                    nc.tensor.matmul(
                        ps[:, sub * 512:sub * 512 + 512],
                        qkT[6 + hp][r0:r0 + D,
                                    bcol + kt * 128:bcol + (kt + 1) * 128],
                        qkT[hp][r0:r0 + D, bcol:bcol + T],
                        start=True, stop=True)
                e = ap_.tile([128, 1024], f16, tag="e")
                nc.scalar.activation(
                    e[:], ps[:], Exp,
                    bias=mask_sb[:, b * 4 + kt:b * 4 + kt + 1],
                    scale=float(SCALE))
                e_tiles.append(e)
            e_hp[(b, hp)] = e_tiles

        def attention_PV(b, hp):
            """PV matmuls + denominator staging for head-pair hp."""
            e_tiles = e_hp.pop((b, hp))
            rs = np_.tile([65, 1024], f16, tag="rstage", bufs=4,
                          name=f"rs{b}_{hp}")
            srow = np_.tile([2, 512], f16, tag="srow", bufs=4,
                            name=f"srow{b}_{hp}")
            srow_tiles[(b, hp)] = srow
            pair = np_.tile([128, 512], f16, tag="pair", bufs=4,
                            name=f"pair{b}_{hp}")
            pair_tiles[(b, hp)] = pair
            for sub in range(2):
                h = 2 * hp + sub
                py = ps_y.tile([128, 512], f32)
                py_tiles[(b, hp, sub)] = py
                for kt in range(4):
                    nc.tensor.matmul(
                        py[:, :],
                        v_t[b * 4 + kt][:, 65 * h:65 * h + 128],
                        e_tiles[kt][:, sub * 512:sub * 512 + 512],
                        start=(kt == 0), stop=(kt == 3))
                # stage softmax denominators (same-base copy, fp16 cast)
                nc.vector.tensor_copy(
                    rs[64:65, sub * 512:sub * 512 + 512], py[64:65, :])
                # relocate to partition `sub` for the K=2 replicate matmul
                nc.sync.dma_start(
                    srow[sub:sub + 1, :],
                    rs[64:65, sub * 512:sub * 512 + 512])
                if sub == 1:
                    st = np_.tile([64, 512], f16, tag="stage")
                    nc.vector.tensor_copy(st[:], py[0:64, :])
                    nc.sync.dma_start(pair[64:128, :], st[:])

        def norm_apply(b, hp, tc_lo=0, tc_hi=4, split=False):
            """Normalize head-pair hp (k-tile hp) of batch b."""
            bcol = b * T
            srow = srow_tiles[(b, hp)]
            if tc_lo == 0:
                rep = ps_mm.tile([128, 512], f32, tag="mm",
                                 name=f"rep{b}_{hp}")
                nc.tensor.matmul(rep[:], e2t[:], srow[:],
                                 start=True, stop=True)
                rcp = np_.tile([128, 512], f32, tag="rcp", bufs=3,
                               name=f"rcp{b}_{hp}")
                srow_tiles[(b, hp, "rcp")] = rcp
                nc.vector.reciprocal_approx_fast(rcp[:], rep[:])
            else:
                rcp = srow_tiles[(b, hp, "rcp")]
            py0 = py_tiles[(b, hp, 0)]
            pair = pair_tiles[(b, hp)]
            lo = tc_lo * 128
            w = (tc_hi - tc_lo) * 128
            nc.vector.tensor_tensor(
                out=yT_t[hp][0:64, bcol + lo:bcol + lo + w],
                in0=py0[0:64, lo:lo + w],
                in1=rcp[0:64, lo:lo + w],
                op=Mult)
            nc.gpsimd.tensor_tensor(
                out=yT_t[hp][64:128, bcol + lo:bcol + lo + w],
                in0=pair[64:128, lo:lo + w],
                in1=rcp[64:128, lo:lo + w],
                op=Mult)

        pj_part = {}
        ot_tiles = {}

        def proj_chunk(b, i, ks=0, ke=KT, partial=False, tail=False):
            t = b * 4 + i // 2
            lo, w = ((0, 512), (512, 256))[i % 2]
            p = ps_mm.tile([128, 512], f32, tag="mm", name=f"pj{b}_{i}_{ks}")
            for k in range(ks, ke):
                nc.tensor.matmul(
                    p[:, :w],
                    yT_t[k][:, t * 128:(t + 1) * 128],
                    wp_t[k][:, lo:lo + w],
                    start=(k == ks), stop=(k == ke - 1 and not partial))
            if partial:
                nc.tensor.matmul(
                    p[:, :w], ones_r[0:1, :], bp16[0:1, lo:lo + w],
                    start=False, stop=True)
                pt = np_.tile([128, 512], f32, tag="pjpart", bufs=8,
                              name=f"pjpart{i}")
                nc.vector.tensor_copy(pt[:, :w], p[:, :w])
                pj_part[(b, i)] = pt
                return
            if tail:
                if i % 2 == 0:
                    ot = np_.tile([128, C], f16, tag="otail", bufs=4,
                                  name=f"ott{i}")
                    ot_tiles[(b, t)] = ot
                else:
                    ot = ot_tiles[(b, t)]
                off = lo
            else:
                if i % 4 == 0:
                    ot = np_.tile([128, 2 * C], f16, tag="ostage", bufs=3,
                                  name=f"ot{b}_{i}")
                    ot_tiles[(b, t // 2)] = ot
                else:
                    ot = ot_tiles[(b, t // 2)]
                off = (t % 2) * C + lo
            if (b, i) in pj_part:
                nc.vector.tensor_tensor(
                    out=ot[:, off:off + w], in0=p[:, :w],
                    in1=pj_part[(b, i)][:, :w], op=Add)
            else:
                nc.vector.tensor_tensor(
                    out=ot[:, off:off + w], in0=p[:, :w],
                    in1=bp_rep[:, lo:lo + w], op=Add)
            if tail:
                q = nc.sync if (i // 2) % 2 == 0 else nc.scalar
                q.dma_start(out_d[t * 128:(t + 1) * 128, lo:lo + w],
                            ot[:, lo:lo + w])
            elif not tail and i % 4 == 3:
                t0 = t - 1
                nc.sync.dma_start(
                    out_d[t0 * 128:(t0 + 2) * 128, :].rearrange(
                        "(t p) c -> p t c", p=128),
                    ot[:].rearrange("p (t c) -> p t c", c=C))

        # ---- software-pipelined emission ----
        # b0: attention is interleaved INTO the qkv chain so exps start
        # during the (HBM-bound) input window. Chain pair (n_hp, n_6+hp)
        # is emitted just before S(b, hp); V-512 chunks early (heads 0-7),
        # V-256 (heads 8-11) later. S leads PV by 2 pairs; norm lags PV
        # by 1 pair.
        B_ITEMS = ([0, 6, 12, 14], [1, 7, 16, 18], [2, 8, 13, 15],
                   [3, 9, 17, 19], [4, 10], [5, 11])
        for i in B_ITEMS[0]:
            qkv_chain(0, i)
        for i in B_ITEMS[1]:
            qkv_chain(0, i)
        attention_S(0, 0)
        attention_S(0, 1)
        nc.sync.dma_start(xt_t[:, 1, :, :], xt_d[:, KT * T:2 * KT * T])
        for hp in range(2, 6):
            if hp == 3:
                nc.sync.dma_start(wp_all[:], wp_d[:, 0:KT * C])
            for i in B_ITEMS[hp]:
                qkv_chain(0, i)
            attention_S(0, hp)
            attention_PV(0, hp - 2)
            if hp >= 3:
                norm_apply(0, hp - 3)
        # drain b0 while feeding the b1 chain
        for g in (0, 1):
            for i in B_ITEMS[g]:
                qkv_chain(1, i)
            attention_PV(0, 4 + g)
            norm_apply(0, 3 + g)
        attention_S(1, 0)
        attention_S(1, 1)
        pj0 = iter(range(8))
        for hp in range(6):
            if hp + 2 <= 5:
                for i in B_ITEMS[hp + 2]:
                    qkv_chain(1, i)
                attention_S(1, hp + 2)
            if hp == 0:
                norm_apply(0, 5)
            if hp == 5:
                # partial-proj first half runs on PE during exp(1,5)
                norm_apply(1, 4)
                for i in range(4):
                    proj_chunk(1, i, 0, 5, partial=True)
            attention_PV(1, hp)
            if 1 <= hp <= 4:
                norm_apply(1, hp - 1)
            for _ in range((1, 1, 1, 1, 2, 2)[hp]):
                i = next(pj0, None)
                if i is not None:
                    proj_chunk(0, i)
        for i in range(4, 8):
            proj_chunk(1, i, 0, 5, partial=True)
        for i in pj0:
            proj_chunk(0, i)
        # pipelined tail: per 128-token chunk, normalize then project k=5
        for tc in range(4):
            norm_apply(1, 5, tc, tc + 1)
            proj_chunk(1, 2 * tc, 5, KT, tail=True)
            proj_chunk(1, 2 * tc + 1, 5, KT, tail=True)

    nc.compile()
    return nc


def get_compiled():
    if "nc" not in _cache:
        _cache["nc"] = _build()
    return _cache["nc"]


def make_in_maps(x, attention_mask, W_attn, b_attn, W_proj, b_proj):
    x = np.asarray(x, dtype=np.float32).astype(np.float16)
    mask = np.asarray(attention_mask, dtype=np.float32)[:, 0, 0, :]
    wa = np.asarray(W_attn, dtype=np.float32).astype(np.float16)
    ba = np.asarray(b_attn, dtype=np.float32)
    wp = np.asarray(W_proj, dtype=np.float32).astype(np.float16)
    bp = np.asarray(b_proj, dtype=np.float32)
    bb = np.ascontiguousarray(
        np.concatenate([ba[2 * C:], bp]).reshape(1, 2 * C))
    # per-partition-contiguous layouts: row p holds that partition's data
    # wqk[p, j, k, c] = W_attn[k*128+p, n*128+c], j = consumption order
    NORD = (0, 6, 1, 7, 2, 8, 3, 9, 4, 10, 5, 11)
    wqk = np.ascontiguousarray(
        wa[:, :2 * C].reshape(KT, 128, NQK, 128).transpose(
            1, 2, 0, 3)[:, NORD].reshape(128, NQK * KT * 128))
    # wv: [p, half0: k x 512 | half1: k x 256]
    wv3 = wa[:, 2 * C:].reshape(KT, 128, C).transpose(1, 0, 2)
    wv = np.ascontiguousarray(np.concatenate(
        [wv3[:, :, 0:512].reshape(128, KT * 512),
         wv3[:, :, 512:C].reshape(128, KT * 256)], axis=1))
    wpp = np.ascontiguousarray(
        wp.reshape(KT, 128, C).transpose(1, 0, 2).reshape(128, KT * C))
    e2 = np.zeros((2, 128), dtype=np.float16)
    e2[0, 0:64] = 1.0
    e2[1, 64:128] = 1.0
    maps = []
    for i in range(N_CORES):
        qm = np.zeros((128, 20), dtype=np.float32)
        qm[:, :NQK] = ba[0:2 * C].reshape(NQK, 128).T
        qm[:, NQK:] = mask[BC * i:BC * (i + 1)].reshape(-1).reshape(
            BC * 4, 128).T
        # xt[p, b, k, m] = x[core, b, m, k*128+p]
        xc = x[BC * i:BC * (i + 1)]                      # [BC, T, C]
        xt = np.ascontiguousarray(
            xc.reshape(BC, T, KT, 128).transpose(3, 0, 2, 1).reshape(
                128, BC * KT * T))
        maps.append({
            "xt": xt, "qm32": qm, "bb32": bb,
            "wqk": wqk, "wv": wv, "wp": wpp, "e2pat": e2,
        })
    return maps


def kernel(x, attention_mask, W_attn, b_attn, W_proj, b_proj):
    from concourse.bass_utils import run_bass_kernel_spmd

    nc = get_compiled()
    in_maps = make_in_maps(x, attention_mask, W_attn, b_attn, W_proj, b_proj)
    last_err = None
    for _ in range(3):
        try:
            res = run_bass_kernel_spmd(nc, in_maps, list(range(N_CORES)))
            break
        except Exception as e:  # transient NRT device errors: retry
            last_err = e
    else:
        raise last_err
    out = np.concatenate(
        [res.results[i]["out"].reshape(BC, T, C) for i in range(N_CORES)],
        axis=0)
    return out.astype(np.float32)
